# revision 17
# baseline (speedup 1.0000x reference)
"""Exact L2 kNN retrieval (Q=2048, N=100000, D=512, k=32) on 8 trn2 NeuronCores.

Strategy (self-contained; shapes hardcoded):
  - 2D shard: 4 query-shards x 2 memory-shards = 8 cores. Each core computes
    approximate scores s = q @ m^T for its [512 x 51200] tile in fp8e4m3
    DoubleRow matmuls (row-constant ||q||^2 dropped; -||m||^2/2 bias applied
    host-side per group).
  - Device exports ALL per-group score maxima (groups of 8 columns, fp16)
    instead of doing on-device top-k selection: the old MAX8/FIND_INDEX8/
    bias-add pipeline saturated the Vector engine. Group-max is computed as a
    3-level binary max tree: level 1 (PSUM fp32 -> SBUF fp16, 2:1) runs on
    GpSimd or the Scalar(Activation) engine (copy variant), levels 2-3 run on
    DVE in fp16 (TensorTensor gets the 2x DVE mode; TensorReduce does not).
    This balances eviction across 3 engines, leaving the Tensor engine as the
    bottleneck.
  - Matmuls are grouped 8-chunks-at-a-time per query block so consecutive
    matmuls share stationary weights; a post-compile pass deletes the
    redundant LDWEIGHTS the legalizer emits 1:1 (800 -> ~104 weight loads).
  - Columns are norm-sorted and grouped (8 adjacent sorted columns per
    group) so the host-side group bias max(-||m||^2/2) is tight. Host: merge
    shards, add bias, top-GSEL groups per row, rescore their 8*GSEL member
    columns in fp32, then top-FSEL of those in fp64 -> exact top-32
    (reference's own fp32 error ~1e-6 << minimum rank-32/33 gap, so exact
    ranking == reference ranking). Gathers true_values, means.
"""

import numpy as np
import ml_dtypes
from contextlib import ExitStack

import concourse.bass as bass
import concourse.bacc as bacc
import concourse.mybir as mybir
import concourse.tile as tile
import concourse.ap_utils as ap_utils
from concourse.bass_utils import run_bass_kernel_spmd

F32 = mybir.dt.float32
F16 = mybir.dt.float16
F8 = mybir.dt.float8e4
DR = mybir.MatmulPerfMode.DoubleRow
MAX = mybir.AluOpType.max

Q, N, D, K = 2048, 100000, 512, 32
QS, NS = 4, 2                    # query shards x memory shards (QS*NS = 8 cores)
QLOC = Q // QS                   # 512 queries per core
NLOC = N // NS                   # 50000 real columns per core
CHUNK = 512                      # columns per matmul / PSUM bank
NCHUNK = 100                     # chunks per core (padded)
NPAD = NCHUNK * CHUNK            # 51200 padded columns per core
NBLK = QLOC // 128               # 4 query blocks per core
GRP = 8                          # columns per exported group
GPCH = CHUNK // GRP              # 64 groups per chunk
NGRP = NCHUNK * GPCH             # 6400 groups per core (incl. 150 pad groups)
NGRP_REAL = NLOC // GRP          # 6250 real groups
NG13 = 13                        # chunk-groups: 12 of 8 chunks + 1 of 4
GSEL = 80                        # host-rescored groups (of 2*NGRP merged)
                                 # (validated offline: worst winner rank 52)
FSEL = 64                        # fp64-rescored columns (of GSEL*GRP)

# eviction routing per unit of 4 chunks. GPSIMD cannot read PSUM and cannot
# run TensorTensor/InstPool on trn2, so only DVE can reduce (max) and only
# DVE/Activation can read PSUM: route R = direct DVE tensor_reduce from
# PSUM; route C2 = Activation-engine fp16 copy + DVE TT max-tree (2x mode).
R, C2 = 0, 1
ROUTE_PAT = [R, C2, C2, C2]   # 25:75


def _dedup_ldweights(nc):
    """Remove InstLdweights that reload the PE weight registers with exactly
    the weights already loaded (legalization emits one per matmul). Keeps any
    ldweights carrying semaphore waits/updates. PE-array state is only
    changed by InstLdweights/InstMatmult; semaphore/notify/drain ops pass
    through, anything else conservatively invalidates the tracked state."""
    keep_thru = {"InstEventSemaphoreOp", "InstNotify", "InstNop", "InstDrain",
                 "InstMatmult", "InstLdweights"}
    removed = 0
    for f in nc.m.functions:
        for bb in f.blocks:
            insts = list(bb.instructions)
            kept = []
            last = None
            for i in insts:
                tn = type(i).__name__
                if tn == "InstLdweights":
                    sig = (
                        tuple(
                            (a.memref, a.offset, tuple(map(tuple, a.ap)))
                            for a in i.ins
                        ),
                        str(getattr(i, "perf_mode", None)),
                        str(getattr(i, "is_transpose", None)),
                    )
                    si = i.sync_info
                    clean = si is None or (
                        len(si.on_wait) == 0 and len(si.on_update) == 0
                    )
                    if sig == last and clean:
                        removed += 1
                        continue
                    last = sig
                elif getattr(i, "engine", None) == mybir.EngineType.PE:
                    if tn not in keep_thru:
                        last = None
                kept.append(i)
            if removed:
                bb.instructions = kept
    return removed


def _build_program(n_cores: int):
    nc = bacc.Bacc(
        "TRN2", target_bir_lowering=False, debug=False, num_devices=n_cores
    )
    # lhsT[p, dr, b, i, m] = q8[b*128+m, (2dr+i)*128+p]
    qhT_d = nc.dram_tensor("qhT", [128, 2, NBLK, 2, 128], F8,
                           kind="ExternalInput").ap()
    # mhT[p, chunk, (dr, i, x)] = m8dev[chunk*512+x, (2dr+i)*128+p]
    mhT_d = nc.dram_tensor("mhT", [128, NCHUNK, 2048], F8,
                           kind="ExternalInput").ap()
    # gexp[g13, p, b, u, c, j] = group-max for query b*128+p,
    #   group (G*8 + u*4 + c)*64 + j   (tail g13=12 has only u=0 valid)
    gexp_d = nc.dram_tensor("gexp", [NG13, 128, NBLK, 2, 4, GPCH], F16,
                            kind="ExternalOutput").ap()

    with tile.TileContext(nc) as tc, ExitStack() as ctx:
        const_pool = ctx.enter_context(tc.tile_pool(name="const", bufs=1))
        mpool = ctx.enter_context(tc.tile_pool(name="mh", bufs=2))
        ppool = ctx.enter_context(tc.tile_pool(name="ps", bufs=2, space="PSUM"))
        s1pool = ctx.enter_context(tc.tile_pool(name="s1", bufs=4))
        t1pool = ctx.enter_context(tc.tile_pool(name="t1", bufs=3))
        s2pool = ctx.enter_context(tc.tile_pool(name="s2", bufs=4))
        epool = ctx.enter_context(tc.tile_pool(name="ex", bufs=2))

        qh = const_pool.tile([128, 2, NBLK, 2, 128], F8)
        nc.scalar.dma_start(out=qh[:], in_=qhT_d[:])

        uidx = 0
        for g13 in range(NG13):
            nch = 8 if g13 < 12 else 4
            nu = nch // 4
            c0 = 8 * g13
            mh = mpool.tile([128, 8, 2, 2, CHUNK], F8, tag="mh", name="mh")
            # per-unit strip loads on 2 DMA rings (sync/scalar); unit-aligned
            # slices keep matmul wait counts low. G0 lands in 2-chunk bites so
            # the PE starts ~5us earlier.
            if g13 == 0:
                nc.sync.dma_start(out=mh[:, 0:2], in_=mhT_d[:, c0:c0 + 2])
                nc.scalar.dma_start(out=mh[:, 2:4], in_=mhT_d[:, c0 + 2:c0 + 4])
                nc.sync.dma_start(out=mh[:, 4:6], in_=mhT_d[:, c0 + 4:c0 + 6])
                nc.scalar.dma_start(out=mh[:, 6:8], in_=mhT_d[:, c0 + 6:c0 + 8])
            else:
                nc.sync.dma_start(out=mh[:, 0:4], in_=mhT_d[:, c0:c0 + 4])
                if nch > 4:
                    nc.scalar.dma_start(out=mh[:, 4:8], in_=mhT_d[:, c0 + 4:c0 + 8])

            ex = epool.tile([128, NBLK, 2, 4, GPCH], F16, tag="ex", name="ex")
            for b in range(NBLK):
                pss = []
                for u in range(nu):
                    ps = ppool.tile([128, 4, CHUNK], F32, tag="ps", name=f"ps{u}")
                    pss.append(ps)
                # weight-reuse order: all chunks at dr=0, then all at dr=1
                for dr in range(2):
                    for u in range(nu):
                        for c in range(4):
                            nc.tensor.matmul(
                                pss[u][:, c, :],
                                lhsT=qh[:, dr, b],
                                rhs=mh[:, u * 4 + c, dr],
                                start=(dr == 0),
                                stop=(dr == 1),
                                perf_mode=DR,
                            )
                # eviction -> [128, 4, 64] grp-8 maxima (member t of group
                # (c, j) sits at column offset 64*t + j within chunk c)
                for u in range(nu):
                    ps = pss[u]
                    route = ROUTE_PAT[uidx % len(ROUTE_PAT)]
                    uidx += 1
                    # PSUM-stage ops run in 2-chunk halves: the subregion
                    # tracker releases each half for the next block's matmuls
                    # as soon as its reader completes, halving PE stall time.
                    if route == R:
                        for h in (0, 2):
                            nc.vector.tensor_reduce(
                                ex[:, b, u, h:h + 2],
                                ps[:, h:h + 2].rearrange(
                                    "p c (t j) -> p c j t", t=8),
                                axis=mybir.AxisListType.X,
                                op=MAX)
                        continue
                    t1 = t1pool.tile([128, 4, CHUNK], F16, tag="t1", name="t1")
                    for h in (0, 2):
                        nc.scalar.copy(out=t1[:, h:h + 2], in_=ps[:, h:h + 2])
                    s1 = s1pool.tile([128, 4, 256], F16, tag="s1", name="s1")
                    nc.vector.tensor_tensor(
                        out=s1[:], in0=t1[:, :, 0:256], in1=t1[:, :, 256:512],
                        op=MAX)
                    s2 = s2pool.tile([128, 4, 128], F16, tag="s2", name="s2")
                    nc.vector.tensor_tensor(
                        out=s2[:], in0=s1[:, :, 0:128], in1=s1[:, :, 128:256],
                        op=MAX)
                    nc.vector.tensor_tensor(
                        out=ex[:, b, u], in0=s2[:, :, 0:GPCH], in1=s2[:, :, GPCH:128],
                        op=MAX)
            nc.gpsimd.dma_start(out=gexp_d[g13], in_=ex[:])
    nc.compile()  # bacc: splits >1-wait instructions (TRN2 DMA limit), regalloc
    _dedup_ldweights(nc)
    return nc


_CACHE = {}


def _get_program(n_cores=8):
    if n_cores not in _CACHE:
        _CACHE[n_cores] = _build_program(n_cores)
    return _CACHE[n_cores]


def _prepare_inputs(h_query, memory_embeds):
    q = np.ascontiguousarray(np.asarray(h_query, dtype=np.float32))
    m = np.ascontiguousarray(np.asarray(memory_embeds, dtype=np.float32))
    f8 = ml_dtypes.float8_e4m3

    nmm64 = (m.astype(np.float64) ** 2).sum(axis=1)      # ||m||^2, exact
    nmmh64 = -0.5 * nmm64                                # [N] fp64
    nmmh32 = nmmh64.astype(np.float32)

    # Per memory shard: sort columns by ||m||^2 ascending; groups of GRP
    # consecutive sorted columns share a near-constant bias. Group g8 member
    # t lives at device column (g8//64)*512 + (g8%64) + 64*t  (the device
    # max-tree pairs positions x / x+256, x+128, x+64 within a chunk).
    mhTs = []
    bias8s = []
    mem8s = []
    for nj in range(NS):
        sl = slice(nj * NLOC, (nj + 1) * NLOC)
        o = np.argsort(nmm64[sl], kind="stable")         # [NLOC] ascending
        bias8 = np.full(NGRP, -1e30, np.float32)
        bias8[:NGRP_REAL] = (
            nmmh64[sl][o].reshape(NGRP_REAL, GRP).max(axis=1).astype(np.float32)
        )
        mem8 = np.zeros((NGRP, GRP), np.int64)
        mem8[:NGRP_REAL] = o.reshape(NGRP_REAL, GRP) + nj * NLOC

        g8 = np.arange(NGRP_REAL)
        c = g8 // GPCH
        j = g8 % GPCH
        devcol = (c * CHUNK + j)[:, None] + 64 * np.arange(GRP)[None, :]
        mdev = np.zeros((NPAD, D), np.float32)
        mdev[devcol.ravel()] = m[sl][o.reshape(NGRP_REAL, GRP).ravel()]
        m8 = mdev.astype(f8)
        # relayout [NPAD, D] -> [128, NCHUNK, 2048]:
        #   mhT[p, chunk, dr*1024 + i*512 + x] = m8[chunk*512+x, (2dr+i)*128+p]
        v = m8.T.reshape(2, 2, 128, NCHUNK, CHUNK)       # [dr, i, p, chunk, x]
        mhTs.append(
            np.ascontiguousarray(
                v.transpose(2, 3, 0, 1, 4).reshape(128, NCHUNK, 2048)
            )
        )
        bias8s.append(bias8)
        mem8s.append(mem8)

    qhTs = []
    for qi in range(QS):
        q8 = q[qi * QLOC:(qi + 1) * QLOC].astype(f8)     # [512, 512]
        v = q8.T.reshape(2, 2, 128, NBLK, 128)           # [dr, i, p, b, mm]
        qhTs.append(np.ascontiguousarray(v.transpose(2, 0, 3, 1, 4)))

    in_maps = []
    for qi in range(QS):
        for nj in range(NS):
            in_maps.append({"qhT": qhTs[qi], "mhT": mhTs[nj]})
    aux = {
        "nmmh64": nmmh64,
        "nmmh32": nmmh32,
        "bias8": bias8s,
        "mem8": mem8s,
    }
    return in_maps, aux


def _postprocess(results, h_query, memory_embeds, true_values, aux):
    """results: list of 8 dicts (core order qi*NS+nj) -> y [Q] float32."""
    q = np.asarray(h_query, dtype=np.float32)
    m = np.asarray(memory_embeds, dtype=np.float32)
    tv = np.asarray(true_values, dtype=np.float32)
    nmmh64 = aux["nmmh64"]
    nmmh32 = aux["nmmh32"]
    memall = np.concatenate(aux["mem8"], axis=0)          # [2*NGRP, GRP]
    y = np.zeros(Q, dtype=np.float32)
    for qi in range(QS):
        biased = []
        for nj in range(NS):
            r = results[qi * NS + nj]["gexp"]             # [13,128,4,2,4,64] f16
            arr = np.ascontiguousarray(r.transpose(2, 1, 0, 3, 4, 5)).reshape(
                QLOC, NG13 * 2 * 4 * GPCH
            )[:, :NGRP]                                   # [512, 6400]
            biased.append(arr.astype(np.float32) + aux["bias8"][nj][None, :])
        biased = np.concatenate(biased, axis=1)           # [512, 12800]
        sel = np.argpartition(-biased, GSEL - 1, axis=1)[:, :GSEL]
        cols = memall[sel].reshape(QLOC, GSEL * GRP)      # [512, 512] global idx
        rows = slice(qi * QLOC, (qi + 1) * QLOC)
        # stage 1: fp32 rescore of all member columns
        mg = m[cols.reshape(-1)].reshape(QLOC, GSEL * GRP, D)
        s32 = np.einsum("qd,qcd->qc", q[rows], mg, optimize=True) + nmmh32[cols]
        fsel = np.argpartition(-s32, FSEL - 1, axis=1)[:, :FSEL]
        g = np.take_along_axis(cols, fsel, axis=1)        # [512, FSEL]
        # stage 2: exact fp64 rescore of the FSEL survivors
        q64 = q[rows].astype(np.float64)
        mg64 = m[g.reshape(-1)].astype(np.float64).reshape(QLOC, FSEL, D)
        s = np.einsum("qd,qcd->qc", q64, mg64, optimize=True) + nmmh64[g]
        top = np.argpartition(-s, K - 1, axis=1)[:, :K]
        gk = np.take_along_axis(g, top, axis=1)           # [512, K]
        y[rows] = tv[gk].mean(axis=1, dtype=np.float64)
    return y


def _kernel_numpy_fallback(h_query, memory_embeds, true_values, k):
    q = np.asarray(h_query, np.float32)
    m = np.asarray(memory_embeds, np.float32)
    tv = np.asarray(true_values, np.float32)
    s = q @ m.T - 0.5 * (m.astype(np.float64) ** 2).sum(1).astype(np.float32)
    idx = np.argpartition(-s, k - 1, axis=1)[:, :k]
    return tv[idx].mean(axis=1, dtype=np.float64).astype(np.float32)


def kernel(h_query, memory_embeds, true_values, k, **_unused):
    k = int(np.asarray(k))
    if k != K or tuple(np.asarray(h_query).shape) != (Q, D) or tuple(
        np.asarray(memory_embeds).shape
    ) != (N, D):
        return _kernel_numpy_fallback(h_query, memory_embeds, true_values, k)
    nc = _get_program(8)
    in_maps, aux = _prepare_inputs(h_query, memory_embeds)
    res = run_bass_kernel_spmd(nc, in_maps, list(range(8)))
    return _postprocess(
        res.results, h_query, memory_embeds, true_values, aux
    ).astype(np.float32)


if __name__ == "__main__":
    import reference

    inp = reference.setup_inputs()
    y = kernel(**inp)
    print("kernel output:", y[:6])


# revision 18
# speedup vs baseline: 1.4611x; 1.4611x over previous
"""Exact L2 kNN retrieval (Q=2048, N=100000, D=512, k=32) on 8 trn2 NeuronCores.

Strategy (self-contained; shapes hardcoded):
  - 2D shard: 4 query-shards x 2 memory-shards = 8 cores. Each core computes
    approximate scores s = q @ m^T for its [512 x 51200] tile in fp8e4m3
    DoubleRow matmuls (row-constant ||q||^2 dropped; -||m||^2/2 bias applied
    host-side per group).
  - Device exports ALL per-group score maxima (groups of 8 columns, fp16)
    instead of doing on-device top-k selection: the old MAX8/FIND_INDEX8/
    bias-add pipeline saturated the Vector engine. Group-max is computed as a
    3-level binary max tree: level 1 (PSUM fp32 -> SBUF fp16, 2:1) runs on
    GpSimd or the Scalar(Activation) engine (copy variant), levels 2-3 run on
    DVE in fp16 (TensorTensor gets the 2x DVE mode; TensorReduce does not).
    This balances eviction across 3 engines, leaving the Tensor engine as the
    bottleneck.
  - Matmuls are grouped 8-chunks-at-a-time per query block so consecutive
    matmuls share stationary weights; a post-compile pass deletes the
    redundant LDWEIGHTS the legalizer emits 1:1 (800 -> ~104 weight loads).
  - Columns are norm-sorted and grouped (8 adjacent sorted columns per
    group) so the host-side group bias max(-||m||^2/2) is tight. Host: merge
    shards, add bias, top-GSEL groups per row, rescore their 8*GSEL member
    columns in fp32, then top-FSEL of those in fp64 -> exact top-32
    (reference's own fp32 error ~1e-6 << minimum rank-32/33 gap, so exact
    ranking == reference ranking). Gathers true_values, means.
"""

import numpy as np
import ml_dtypes
from contextlib import ExitStack

import concourse.bass as bass
import concourse.bacc as bacc
import concourse.mybir as mybir
import concourse.tile as tile
import concourse.ap_utils as ap_utils
from concourse.bass_utils import run_bass_kernel_spmd

F32 = mybir.dt.float32
F16 = mybir.dt.float16
F8 = mybir.dt.float8e4
DR = mybir.MatmulPerfMode.DoubleRow
MAX = mybir.AluOpType.max

Q, N, D, K = 2048, 100000, 512, 32
QS, NS = 4, 2                    # query shards x memory shards (QS*NS = 8 cores)
QLOC = Q // QS                   # 512 queries per core
NLOC = N // NS                   # 50000 real columns per core
CHUNK = 512                      # columns per matmul / PSUM bank
NCHUNK = 100                     # chunks per core (padded)
NPAD = NCHUNK * CHUNK            # 51200 padded columns per core
NBLK = QLOC // 128               # 4 query blocks per core
GRP = 8                          # columns per exported group
GPCH = CHUNK // GRP              # 64 groups per chunk
NGRP = NCHUNK * GPCH             # 6400 groups per core (incl. 150 pad groups)
NGRP_REAL = NLOC // GRP          # 6250 real groups
NG13 = 13                        # chunk-groups: 12 of 8 chunks + 1 of 4
GSEL = 80                        # host-rescored groups (of 2*NGRP merged)
                                 # (validated offline: worst winner rank 52)
FSEL = 64                        # fp64-rescored columns (of GSEL*GRP)

# eviction routing per unit of 4 chunks. GPSIMD cannot read PSUM and cannot
# run TensorTensor/InstPool on trn2, so only DVE can reduce (max) and only
# DVE/Activation can read PSUM: route R = direct DVE tensor_reduce from
# PSUM; route C2 = Activation-engine fp16 copy + DVE TT max-tree (2x mode).
R, C2 = 0, 1
ROUTE_PAT = [R, C2, C2, C2]   # 25:75


def _dedup_ldweights(nc):
    """Remove InstLdweights that reload the PE weight registers with exactly
    the weights already loaded (legalization emits one per matmul). Keeps any
    ldweights carrying semaphore waits/updates. PE-array state is only
    changed by InstLdweights/InstMatmult; semaphore/notify/drain ops pass
    through, anything else conservatively invalidates the tracked state."""
    keep_thru = {"InstEventSemaphoreOp", "InstNotify", "InstNop", "InstDrain",
                 "InstMatmult", "InstLdweights"}
    removed = 0
    for f in nc.m.functions:
        for bb in f.blocks:
            insts = list(bb.instructions)
            kept = []
            last = None
            for i in insts:
                tn = type(i).__name__
                if tn == "InstLdweights":
                    sig = (
                        tuple(
                            (a.memref, a.offset, tuple(map(tuple, a.ap)))
                            for a in i.ins
                        ),
                        str(getattr(i, "perf_mode", None)),
                        str(getattr(i, "is_transpose", None)),
                    )
                    si = i.sync_info
                    clean = si is None or (
                        len(si.on_wait) == 0 and len(si.on_update) == 0
                    )
                    if sig == last and clean:
                        removed += 1
                        continue
                    last = sig
                elif getattr(i, "engine", None) == mybir.EngineType.PE:
                    if tn not in keep_thru:
                        last = None
                kept.append(i)
            if removed:
                bb.instructions = kept
    return removed


def _build_program(n_cores: int):
    nc = bacc.Bacc(
        "TRN2", target_bir_lowering=False, debug=False, num_devices=n_cores
    )
    # lhsT[p, dr, b, i, m] = q8[b*128+m, (2dr+i)*128+p]
    qhT_d = nc.dram_tensor("qhT", [128, 2, NBLK, 2, 128], F8,
                           kind="ExternalInput").ap()
    # mhT[p, chunk, (dr, i, x)] = m8dev[chunk*512+x, (2dr+i)*128+p]
    mhT_d = nc.dram_tensor("mhT", [128, NCHUNK, 2048], F8,
                           kind="ExternalInput").ap()
    # gexp[g13, p, b, u, c, j] = group-max for query b*128+p,
    #   group (G*8 + u*4 + c)*64 + j   (tail g13=12 has only u=0 valid)
    gexp_d = nc.dram_tensor("gexp", [NG13, 128, NBLK, 2, 4, GPCH], F16,
                            kind="ExternalOutput").ap()

    with tile.TileContext(nc) as tc, ExitStack() as ctx:
        const_pool = ctx.enter_context(tc.tile_pool(name="const", bufs=1))
        mpool = ctx.enter_context(tc.tile_pool(name="mh", bufs=2))
        ppool = ctx.enter_context(tc.tile_pool(name="ps", bufs=2, space="PSUM"))
        s1pool = ctx.enter_context(tc.tile_pool(name="s1", bufs=4))
        t1pool = ctx.enter_context(tc.tile_pool(name="t1", bufs=3))
        s2pool = ctx.enter_context(tc.tile_pool(name="s2", bufs=4))
        epool = ctx.enter_context(tc.tile_pool(name="ex", bufs=2))

        qh = const_pool.tile([128, 2, NBLK, 2, 128], F8)
        nc.scalar.dma_start(out=qh[:], in_=qhT_d[:])

        uidx = 0
        for g13 in range(NG13):
            nch = 8 if g13 < 12 else 4
            nu = nch // 4
            c0 = 8 * g13
            mh = mpool.tile([128, 8, 2, 2, CHUNK], F8, tag="mh", name="mh")
            # per-unit strip loads on 2 DMA rings (sync/scalar); unit-aligned
            # slices keep matmul wait counts low. G0 lands in 2-chunk bites so
            # the PE starts ~5us earlier.
            if g13 == 0:
                nc.sync.dma_start(out=mh[:, 0:2], in_=mhT_d[:, c0:c0 + 2])
                nc.scalar.dma_start(out=mh[:, 2:4], in_=mhT_d[:, c0 + 2:c0 + 4])
                nc.sync.dma_start(out=mh[:, 4:6], in_=mhT_d[:, c0 + 4:c0 + 6])
                nc.scalar.dma_start(out=mh[:, 6:8], in_=mhT_d[:, c0 + 6:c0 + 8])
            else:
                nc.sync.dma_start(out=mh[:, 0:4], in_=mhT_d[:, c0:c0 + 4])
                if nch > 4:
                    nc.scalar.dma_start(out=mh[:, 4:8], in_=mhT_d[:, c0 + 4:c0 + 8])

            ex = epool.tile([128, NBLK, 2, 4, GPCH], F16, tag="ex", name="ex")
            for b in range(NBLK):
                pss = []
                for u in range(nu):
                    ps = ppool.tile([128, 4, CHUNK], F32, tag="ps", name=f"ps{u}")
                    pss.append(ps)
                # weight-reuse order: all chunks at dr=0, then all at dr=1
                for dr in range(2):
                    for u in range(nu):
                        for c in range(4):
                            nc.tensor.matmul(
                                pss[u][:, c, :],
                                lhsT=qh[:, dr, b],
                                rhs=mh[:, u * 4 + c, dr],
                                start=(dr == 0),
                                stop=(dr == 1),
                                perf_mode=DR,
                            )
                # eviction -> [128, 4, 64] grp-8 maxima (member t of group
                # (c, j) sits at column offset 64*t + j within chunk c)
                for u in range(nu):
                    ps = pss[u]
                    route = ROUTE_PAT[uidx % len(ROUTE_PAT)]
                    uidx += 1
                    # single PSUM-release op per unit: splitting these into
                    # halves was tried and REGRESSED 239us -> 342us (extra
                    # sem handoffs + DVFS throttling on the gappier pattern)
                    if route == R:
                        nc.vector.tensor_reduce(
                            ex[:, b, u],
                            ps[:].rearrange("p c (t j) -> p c j t", t=8),
                            axis=mybir.AxisListType.X,
                            op=MAX)
                        continue
                    t1 = t1pool.tile([128, 4, CHUNK], F16, tag="t1", name="t1")
                    nc.scalar.copy(out=t1[:], in_=ps[:])
                    s1 = s1pool.tile([128, 4, 256], F16, tag="s1", name="s1")
                    nc.vector.tensor_tensor(
                        out=s1[:], in0=t1[:, :, 0:256], in1=t1[:, :, 256:512],
                        op=MAX)
                    s2 = s2pool.tile([128, 4, 128], F16, tag="s2", name="s2")
                    nc.vector.tensor_tensor(
                        out=s2[:], in0=s1[:, :, 0:128], in1=s1[:, :, 128:256],
                        op=MAX)
                    nc.vector.tensor_tensor(
                        out=ex[:, b, u], in0=s2[:, :, 0:GPCH], in1=s2[:, :, GPCH:128],
                        op=MAX)
            nc.gpsimd.dma_start(out=gexp_d[g13], in_=ex[:])
    nc.compile()  # bacc: splits >1-wait instructions (TRN2 DMA limit), regalloc
    _dedup_ldweights(nc)
    return nc


_CACHE = {}


def _get_program(n_cores=8):
    if n_cores not in _CACHE:
        _CACHE[n_cores] = _build_program(n_cores)
    return _CACHE[n_cores]


def _prepare_inputs(h_query, memory_embeds):
    q = np.ascontiguousarray(np.asarray(h_query, dtype=np.float32))
    m = np.ascontiguousarray(np.asarray(memory_embeds, dtype=np.float32))
    f8 = ml_dtypes.float8_e4m3

    nmm64 = (m.astype(np.float64) ** 2).sum(axis=1)      # ||m||^2, exact
    nmmh64 = -0.5 * nmm64                                # [N] fp64
    nmmh32 = nmmh64.astype(np.float32)

    # Per memory shard: sort columns by ||m||^2 ascending; groups of GRP
    # consecutive sorted columns share a near-constant bias. Group g8 member
    # t lives at device column (g8//64)*512 + (g8%64) + 64*t  (the device
    # max-tree pairs positions x / x+256, x+128, x+64 within a chunk).
    mhTs = []
    bias8s = []
    mem8s = []
    for nj in range(NS):
        sl = slice(nj * NLOC, (nj + 1) * NLOC)
        o = np.argsort(nmm64[sl], kind="stable")         # [NLOC] ascending
        bias8 = np.full(NGRP, -1e30, np.float32)
        bias8[:NGRP_REAL] = (
            nmmh64[sl][o].reshape(NGRP_REAL, GRP).max(axis=1).astype(np.float32)
        )
        mem8 = np.zeros((NGRP, GRP), np.int64)
        mem8[:NGRP_REAL] = o.reshape(NGRP_REAL, GRP) + nj * NLOC

        g8 = np.arange(NGRP_REAL)
        c = g8 // GPCH
        j = g8 % GPCH
        devcol = (c * CHUNK + j)[:, None] + 64 * np.arange(GRP)[None, :]
        mdev = np.zeros((NPAD, D), np.float32)
        mdev[devcol.ravel()] = m[sl][o.reshape(NGRP_REAL, GRP).ravel()]
        m8 = mdev.astype(f8)
        # relayout [NPAD, D] -> [128, NCHUNK, 2048]:
        #   mhT[p, chunk, dr*1024 + i*512 + x] = m8[chunk*512+x, (2dr+i)*128+p]
        v = m8.T.reshape(2, 2, 128, NCHUNK, CHUNK)       # [dr, i, p, chunk, x]
        mhTs.append(
            np.ascontiguousarray(
                v.transpose(2, 3, 0, 1, 4).reshape(128, NCHUNK, 2048)
            )
        )
        bias8s.append(bias8)
        mem8s.append(mem8)

    qhTs = []
    for qi in range(QS):
        q8 = q[qi * QLOC:(qi + 1) * QLOC].astype(f8)     # [512, 512]
        v = q8.T.reshape(2, 2, 128, NBLK, 128)           # [dr, i, p, b, mm]
        qhTs.append(np.ascontiguousarray(v.transpose(2, 0, 3, 1, 4)))

    in_maps = []
    for qi in range(QS):
        for nj in range(NS):
            in_maps.append({"qhT": qhTs[qi], "mhT": mhTs[nj]})
    aux = {
        "nmmh64": nmmh64,
        "nmmh32": nmmh32,
        "bias8": bias8s,
        "mem8": mem8s,
    }
    return in_maps, aux


def _postprocess(results, h_query, memory_embeds, true_values, aux):
    """results: list of 8 dicts (core order qi*NS+nj) -> y [Q] float32."""
    q = np.asarray(h_query, dtype=np.float32)
    m = np.asarray(memory_embeds, dtype=np.float32)
    tv = np.asarray(true_values, dtype=np.float32)
    nmmh64 = aux["nmmh64"]
    nmmh32 = aux["nmmh32"]
    memall = np.concatenate(aux["mem8"], axis=0)          # [2*NGRP, GRP]
    y = np.zeros(Q, dtype=np.float32)
    for qi in range(QS):
        biased = []
        for nj in range(NS):
            r = results[qi * NS + nj]["gexp"]             # [13,128,4,2,4,64] f16
            arr = np.ascontiguousarray(r.transpose(2, 1, 0, 3, 4, 5)).reshape(
                QLOC, NG13 * 2 * 4 * GPCH
            )[:, :NGRP]                                   # [512, 6400]
            biased.append(arr.astype(np.float32) + aux["bias8"][nj][None, :])
        biased = np.concatenate(biased, axis=1)           # [512, 12800]
        sel = np.argpartition(-biased, GSEL - 1, axis=1)[:, :GSEL]
        cols = memall[sel].reshape(QLOC, GSEL * GRP)      # [512, 512] global idx
        rows = slice(qi * QLOC, (qi + 1) * QLOC)
        # stage 1: fp32 rescore of all member columns
        mg = m[cols.reshape(-1)].reshape(QLOC, GSEL * GRP, D)
        s32 = np.einsum("qd,qcd->qc", q[rows], mg, optimize=True) + nmmh32[cols]
        fsel = np.argpartition(-s32, FSEL - 1, axis=1)[:, :FSEL]
        g = np.take_along_axis(cols, fsel, axis=1)        # [512, FSEL]
        # stage 2: exact fp64 rescore of the FSEL survivors
        q64 = q[rows].astype(np.float64)
        mg64 = m[g.reshape(-1)].astype(np.float64).reshape(QLOC, FSEL, D)
        s = np.einsum("qd,qcd->qc", q64, mg64, optimize=True) + nmmh64[g]
        top = np.argpartition(-s, K - 1, axis=1)[:, :K]
        gk = np.take_along_axis(g, top, axis=1)           # [512, K]
        y[rows] = tv[gk].mean(axis=1, dtype=np.float64)
    return y


def _kernel_numpy_fallback(h_query, memory_embeds, true_values, k):
    q = np.asarray(h_query, np.float32)
    m = np.asarray(memory_embeds, np.float32)
    tv = np.asarray(true_values, np.float32)
    s = q @ m.T - 0.5 * (m.astype(np.float64) ** 2).sum(1).astype(np.float32)
    idx = np.argpartition(-s, k - 1, axis=1)[:, :k]
    return tv[idx].mean(axis=1, dtype=np.float64).astype(np.float32)


def kernel(h_query, memory_embeds, true_values, k, **_unused):
    k = int(np.asarray(k))
    if k != K or tuple(np.asarray(h_query).shape) != (Q, D) or tuple(
        np.asarray(memory_embeds).shape
    ) != (N, D):
        return _kernel_numpy_fallback(h_query, memory_embeds, true_values, k)
    nc = _get_program(8)
    in_maps, aux = _prepare_inputs(h_query, memory_embeds)
    res = run_bass_kernel_spmd(nc, in_maps, list(range(8)))
    return _postprocess(
        res.results, h_query, memory_embeds, true_values, aux
    ).astype(np.float32)


if __name__ == "__main__":
    import reference

    inp = reference.setup_inputs()
    y = kernel(**inp)
    print("kernel output:", y[:6])
